# revision 26
# baseline (speedup 1.0000x reference)
"""AdaptiveTokenSampling Trainium2 kernel (8 NeuronCores, data-parallel over batch).

reference semantics (per batch b):
  cls_attn = attn[b,:,0,1:]                      [h, n-1]
  norms    = ||value[b,:,1:,:]||_2 over d        [h, n-1]
  scores   = sum_h cls_attn * norms              [n-1]
  normed   = scores / (sum(scores)+eps)
  pl       = log(normed+eps)  (masked -> -big)
  logits   = pl + (-log(-log(u+eps)+eps))        [k, n-1]
  sampled  = argmax_n logits + 1                 [k]
  uniq-sort -> ids [k+1] (CLS 0 first, zeros pad in front of ascending uniques)
  new_attn = attn[b,:,ids,:], new_mask = ids!=0 (True at 0)

device impl notes:
  argmax_n (pl + gumbel) == argmax_n (normed+eps)/X  with X = -log(u+eps)+eps
  (monotone transform; avoids two of the three logs).
  sort/unique via presence bitmap over tokens 1..1024 + matmul cumsum ranks +
  one-hot matmul scatter into output slots.
  row gather via gpsimd indirect DMA (DRAM->SBUF) + plain store.
"""

import os
import sys

for _p in ("/opt/trn_rl_repo", "/root/.axon_site/_ro/trn_rl_repo"):
    if _p not in sys.path:
        sys.path.append(_p)

import numpy as np

B, H, N, D, K = 16, 12, 1025, 64, 256
NM1 = N - 1          # 1024 tokens (excl CLS)
KP1 = K + 1          # 257 output slots
EPS = 1e-6
NCORES = 8
BPC = B // NCORES    # 2 batch elements per core
NT = NM1 // 128      # 8 token tiles
P = 128

_BUILT = {}


def _build():
    if "nc" in _BUILT:
        return _BUILT["nc"]
    import concourse.bacc as bacc
    import concourse.bass as bass
    import concourse.mybir as mybir
    from concourse.tile import TileContext

    f32 = mybir.dt.float32
    i32 = mybir.dt.int32
    u8 = mybir.dt.uint8
    AF = mybir.ActivationFunctionType
    OP = mybir.AluOpType
    AX = mybir.AxisListType

    KSTAGE = int(os.environ.get("KSTAGE", "99"))
    nc = bacc.Bacc("TRN2", target_bir_lowering=False, debug=False)

    attn_d = nc.dram_tensor("attn", [BPC, H, N, N], f32, kind="ExternalInput")
    value_d = nc.dram_tensor("value", [BPC, H, N, D], f32, kind="ExternalInput")
    gum_d = nc.dram_tensor("gum", [BPC, K, NM1], f32, kind="ExternalInput")
    mask_d = nc.dram_tensor("mask", [BPC, N], u8, kind="ExternalInput")

    nattn_d = nc.dram_tensor("nattn", [BPC, H, KP1, N], f32, kind="ExternalOutput")
    ids_d = nc.dram_tensor("ids", [BPC, KP1], i32, kind="ExternalOutput")
    nmask_d = nc.dram_tensor("nmask", [BPC, KP1], u8, kind="ExternalOutput")

    KDEBUG = bool(int(os.environ.get("KDEBUG", "0")))
    if KDEBUG:
        dbg_arow = nc.dram_tensor("dbg_arow", [BPC, NM1], f32, kind="ExternalOutput")
        dbg_samp = nc.dram_tensor("dbg_samp", [BPC, 2, P], f32, kind="ExternalOutput")
        dbg_pres = nc.dram_tensor("dbg_pres", [BPC, P, NT], f32, kind="ExternalOutput")
        dbg_slot = nc.dram_tensor("dbg_slot", [BPC, P, NT], f32, kind="ExternalOutput")
        dbg_nrm = nc.dram_tensor("dbg_nrm", [BPC, P, NT * H], f32, kind="ExternalOutput")
        dbg_cls = nc.dram_tensor("dbg_cls", [BPC, P, NT * H], f32, kind="ExternalOutput")
        dbg_cnt = nc.dram_tensor("dbg_cnt", [BPC, P, NT], f32, kind="ExternalOutput")
        dbg_oh = nc.dram_tensor("dbg_oh", [BPC, 2, P, NM1], f32, kind="ExternalOutput")

    # ---- inline constants
    iota_bc_np = np.broadcast_to(np.arange(NM1, dtype=np.float32), (P, NM1)).copy()
    iota257_np = np.broadcast_to(np.arange(KP1, dtype=np.float32), (P, KP1)).copy()
    ncol_np = (np.arange(NT)[None, :] * P + np.arange(P)[:, None] + 1).astype(np.float32)
    u128_np = np.triu(np.ones((P, P), np.float32))          # U[q,p]=1 if q<=p
    # su9[s,t] = 1 if s < t, for t in 0..8; col 8 (t=NT) = all ones = total count
    su9_np = np.zeros((NT, NT + 1), np.float32)
    for s in range(NT):
        su9_np[s, s + 1:] = 1.0
    iota_bc_d = nc.inline_tensor(iota_bc_np, name="iota_bc")
    iota257_d = nc.inline_tensor(iota257_np, name="iota257")
    ncol_d = nc.inline_tensor(ncol_np, name="ncol")
    u128_d = nc.inline_tensor(u128_np, name="u128")
    su9_d = nc.inline_tensor(su9_np, name="su9")
    e0row_np = np.zeros((1, KP1), np.float32); e0row_np[0, 0] = 1.0
    e0row_d = nc.inline_tensor(e0row_np, name="e0row")  # 1 at col 0
    ident_d = nc.inline_tensor(np.eye(P, dtype=np.float32), name="ident")
    ones128_d = nc.inline_tensor(np.ones((P, 1), np.float32), name="ones128")
    ones1x128_d = nc.inline_tensor(np.ones((1, P), np.float32), name="ones1x128")
    epscol_d = nc.inline_tensor(np.full((P, 1), EPS, np.float32), name="epscol")

    attn_rows = attn_d[:].rearrange("a h n m -> (a h n) m")  # [BPC*H*N, N] for gather

    with TileContext(nc) as tc:
        with tc.tile_pool(name="consts", bufs=1) as cpool, \
             tc.tile_pool(name="inp", bufs=3) as ipool, \
             tc.tile_pool(name="mid", bufs=2) as mpool, \
             tc.tile_pool(name="small", bufs=2) as spool, \
             tc.tile_pool(name="gath", bufs=6) as gpool, \
             tc.tile_pool(name="ps_tp", bufs=1, space="PSUM") as ps_tp, \
             tc.tile_pool(name="ps_abc", bufs=1, space="PSUM") as ps_abc, \
             tc.tile_pool(name="ps_cnt", bufs=1, space="PSUM") as ps_cnt, \
             tc.tile_pool(name="ps_rank", bufs=1, space="PSUM") as ps_rank, \
             tc.tile_pool(name="ps_tiny", bufs=1, space="PSUM") as ps_tiny, \
             tc.tile_pool(name="ps_row", bufs=2, space="PSUM") as ps_row, \
             tc.tile_pool(name="ps_ids", bufs=1, space="PSUM") as ps_ids:

            iota_bc = cpool.tile([P, NM1], f32)
            nc.sync.dma_start(out=iota_bc[:], in_=iota_bc_d[:])
            iota257 = cpool.tile([P, KP1], f32)
            nc.sync.dma_start(out=iota257[:], in_=iota257_d[:])
            ncol = cpool.tile([P, NT], f32)
            nc.sync.dma_start(out=ncol[:], in_=ncol_d[:])
            u128 = cpool.tile([P, P], f32)
            nc.sync.dma_start(out=u128[:], in_=u128_d[:])
            su9 = cpool.tile([NT, NT + 1], f32)
            nc.sync.dma_start(out=su9[:], in_=su9_d[:])
            e0row = cpool.tile([1, KP1], f32)
            nc.sync.dma_start(out=e0row[:], in_=e0row_d[:])
            ident = cpool.tile([P, P], f32)
            nc.sync.dma_start(out=ident[:], in_=ident_d[:])
            ones128 = cpool.tile([P, 1], f32)
            nc.sync.dma_start(out=ones128[:], in_=ones128_d[:])
            ones1x128 = cpool.tile([1, P], f32)
            nc.sync.dma_start(out=ones1x128[:], in_=ones1x128_d[:])
            epscol = cpool.tile([P, 1], f32)
            nc.sync.dma_start(out=epscol[:], in_=epscol_d[:])

            for b in range(BPC):
                if KSTAGE < 1:
                    break
                # ---------- stage 1: norms over d for each head ----------
                # norms2_all[p, t, h] layout: [128, NT*H] with h innermost
                n2 = spool.tile([P, NT * H], f32, tag="n2")
                n2v = n2[:].rearrange("p (t h) -> p t h", t=NT)
                for h in range(H):
                    vt = ipool.tile([P, NT * D], f32, tag="vt")
                    nc.sync.dma_start(
                        out=vt[:],
                        in_=value_d[b, h, 1:, :].rearrange("(t p) d -> p t d", p=P),
                    )
                    sq = ipool.tile([P, NT * D], f32, tag="sq")
                    nc.scalar.activation(sq[:], vt[:], AF.Square)
                    nc.vector.tensor_reduce(
                        out=n2v[:, :, h],
                        in_=sq[:].rearrange("p (t d) -> p t d", t=NT),
                        axis=AX.X, op=OP.add,
                    )
                # norms = sqrt(n2) with one Newton refinement (hw sqrt ~7e-6 rel)
                nrm0 = spool.tile([P, NT * H], f32, tag="nrm0")
                nc.scalar.activation(nrm0[:], n2[:], AF.Sqrt)
                nrec = spool.tile([P, NT * H], f32, tag="nrec")
                nc.vector.reciprocal(nrec[:], nrm0[:])
                nxt = spool.tile([P, NT * H], f32, tag="nxt")
                nc.vector.tensor_tensor(out=nxt[:], in0=n2[:], in1=nrec[:], op=OP.mult)
                nrm = spool.tile([P, NT * H], f32, tag="nrm")
                nc.vector.tensor_tensor(out=nrm[:], in0=nrm0[:], in1=nxt[:], op=OP.add)
                nc.vector.tensor_scalar_mul(nrm[:], nrm[:], 0.5)

                if KSTAGE < 2:
                    continue
                # ---------- stage 2: cls_attn -> [128, NT*H] via transposes ----------
                cls = ipool.tile([H, NM1], f32, tag="cls")
                nc.sync.dma_start(out=cls[:], in_=attn_d[b, :, 0, 1:])
                cls_all = spool.tile([P, NT * H], f32, tag="cls_all")
                cls_allv = cls_all[:].rearrange("p (t h) -> p t h", t=NT)
                for t in range(NT):
                    clsT_ps = ps_tp.tile([P, H], f32, tag="tp", space="PSUM")
                    nc.tensor.transpose(
                        out=clsT_ps[:], in_=cls[:, t * P:(t + 1) * P],
                        identity=ident[:H, :H],
                    )
                    nc.vector.tensor_copy(out=cls_allv[:, t, :], in_=clsT_ps[:])

                if KSTAGE < 3:
                    continue
                # ---------- stage 3: scores ----------
                prod = spool.tile([P, NT * H], f32, tag="prod")
                nc.vector.tensor_tensor(out=prod[:], in0=cls_all[:], in1=nrm[:], op=OP.mult)
                scores = spool.tile([P, NT], f32, tag="scores")
                nc.vector.tensor_reduce(
                    out=scores[:], in_=prod[:].rearrange("p (t h) -> p t h", t=NT),
                    axis=AX.X, op=OP.add,
                )
                # transpose [128, NT] -> [NT, 128] -> reshape row [1, 1024]
                scT_ps = ps_tp.tile([NT, P], f32, tag="tp", space="PSUM")
                nc.tensor.transpose(out=scT_ps[:], in_=scores[:], identity=ident[:])
                scT = spool.tile([NT, P], f32, tag="scT_sb")
                nc.vector.tensor_copy(out=scT[:], in_=scT_ps[:])
                srow = spool.tile([1, NM1], f32, tag="srow")
                nc.sync.dma_start(
                    out=srow[0:1, :].rearrange("o (t q) -> o t q", t=NT), in_=scT[:]
                )
                # A = normed + eps (times mask)
                ssum = spool.tile([1, 1], f32, tag="ssum")
                nc.vector.tensor_reduce(out=ssum[:], in_=srow[:], axis=AX.X, op=OP.add)
                nc.vector.tensor_scalar_add(ssum[:], ssum[:], EPS)
                srec = spool.tile([1, 1], f32, tag="srec")
                nc.vector.reciprocal(srec[:], ssum[:])
                arow = spool.tile([1, NM1], f32, tag="arow")
                nc.vector.tensor_scalar(
                    out=arow[:], in0=srow[:], scalar1=srec[:, 0:1], scalar2=EPS,
                    op0=OP.mult, op1=OP.add,
                )
                mrow_u8 = spool.tile([1, NM1], u8, tag="mrow_u8")
                nc.sync.dma_start(out=mrow_u8[:], in_=mask_d[b:b + 1, 1:])
                mrow = spool.tile([1, NM1], f32, tag="mrow")
                nc.vector.tensor_copy(out=mrow[:], in_=mrow_u8[:])
                nc.vector.tensor_tensor(out=arow[:], in0=arow[:], in1=mrow[:], op=OP.mult)
                if KDEBUG:
                    nc.sync.dma_start(out=dbg_arow[b:b + 1, :], in_=arow[:])
                    nc.sync.dma_start(out=dbg_nrm[b], in_=nrm[:])
                    nc.sync.dma_start(out=dbg_cls[b], in_=cls_all[:])
                # broadcast A to [128, 1024]
                abc = mpool.tile([P, NM1], f32, tag="abc")
                for half in range(2):
                    abc_ps = ps_abc.tile([P, NM1 // 2], f32, tag="abc_ps", space="PSUM")
                    nc.tensor.matmul(
                        out=abc_ps[:], lhsT=ones1x128[:],
                        rhs=arow[:, half * (NM1 // 2):(half + 1) * (NM1 // 2)],
                        start=True, stop=True,
                    )
                    nc.vector.tensor_copy(
                        out=abc[:, half * (NM1 // 2):(half + 1) * (NM1 // 2)], in_=abc_ps[:]
                    )

                if KSTAGE < 4:
                    continue
                # ---------- stage 4: gumbel ratio argmax ----------
                onehots = []
                for kt in range(2):
                    gum = mpool.tile([P, NM1], f32, tag="gum")
                    nc.sync.dma_start(out=gum[:], in_=gum_d[b, kt * P:(kt + 1) * P, :])
                    lnx = mpool.tile([P, NM1], f32, tag="lnx")
                    nc.scalar.activation(lnx[:], gum[:], AF.Ln, bias=epscol[:, 0:1])
                    # X = -ln(u+eps) + eps
                    nc.vector.tensor_scalar(
                        out=lnx[:], in0=lnx[:], scalar1=-1.0, scalar2=EPS,
                        op0=OP.mult, op1=OP.add,
                    )
                    rx = mpool.tile([P, NM1], f32, tag="rx")
                    nc.vector.reciprocal(rx[:], lnx[:])
                    nc.vector.tensor_tensor(out=rx[:], in0=rx[:], in1=abc[:], op=OP.mult)
                    m8 = spool.tile([P, 8], f32, tag="m8")
                    nc.vector.max(out=m8[:], in_=rx[:])
                    i8 = spool.tile([P, 8], mybir.dt.uint32, tag="i8")
                    nc.vector.max_index(i8[:], m8[:], rx[:])
                    samp = spool.tile([P, 1], f32, tag="samp")
                    nc.vector.tensor_copy(out=samp[:], in_=i8[:, 0:1])
                    if KDEBUG:
                        nc.sync.dma_start(out=dbg_samp[b, kt, :, None], in_=samp[:])
                    onehot = mpool.tile([P, NM1], f32, tag=f"onehot{kt}")
                    nc.vector.tensor_scalar(
                        out=onehot[:], in0=iota_bc[:], scalar1=samp[:, 0:1], scalar2=None,
                        op0=OP.is_equal,
                    )
                    if KDEBUG:
                        nc.sync.dma_start(out=dbg_oh[b, kt], in_=onehot[:])
                    onehots.append(onehot)

                # count per token as a row [1, 1024]: single-shot matmuls only
                ohsum = mpool.tile([P, NM1], f32, tag="ohsum")
                nc.vector.tensor_tensor(out=ohsum[:], in0=onehots[0][:], in1=onehots[1][:], op=OP.add)
                cnt_row = spool.tile([1, NM1], f32, tag="cnt_row")
                for half in range(2):
                    cr_ps = ps_row.tile([1, NM1 // 2], f32, tag="cr", space="PSUM")
                    nc.tensor.matmul(
                        out=cr_ps[:], lhsT=ones128[:],
                        rhs=ohsum[:, half * (NM1 // 2):(half + 1) * (NM1 // 2)],
                        start=True, stop=True,
                    )
                    nc.vector.tensor_copy(out=cnt_row[:, half * (NM1 // 2):(half + 1) * (NM1 // 2)], in_=cr_ps[:])
                # reshape row -> [128, NT] columns (token n0 = t*128 + p)
                cnt_col = spool.tile([P, NT], f32, tag="cnt_col")
                for t in range(NT):
                    nc.sync.dma_start(
                        out=cnt_col[:, t:t + 1], in_=cnt_row[0:1, t * P:(t + 1) * P]
                    )

                if KSTAGE < 5:
                    continue
                # ---------- stage 5: presence, ranks, slots ----------
                if KDEBUG:
                    nc.sync.dma_start(out=dbg_cnt[b], in_=cnt_col[:])
                pres = spool.tile([P, NT], f32, tag="pres")
                nc.vector.tensor_scalar(
                    out=pres[:], in0=cnt_col[:], scalar1=0.5, scalar2=None, op0=OP.is_ge
                )
                val = spool.tile([P, NT], f32, tag="val")
                nc.vector.tensor_tensor(out=val[:], in0=pres[:], in1=ncol[:], op=OP.mult)

                cum_ps = ps_rank.tile([P, NT], f32, tag="cum", space="PSUM")
                nc.tensor.matmul(out=cum_ps[:], lhsT=u128[:], rhs=pres[:], start=True, stop=True)
                tot_ps = ps_tiny.tile([NT, 1], f32, tag="tiny", space="PSUM")
                nc.tensor.matmul(out=tot_ps[:], lhsT=pres[:], rhs=ones128[:], start=True, stop=True)
                tot = spool.tile([NT, 1], f32, tag="tot_sb")
                nc.vector.tensor_copy(out=tot[:], in_=tot_ps[:])
                # carry_ext[0, t] = sum_{s<t} tot[s] for t<8; col 8 = c (total uniques)
                carry_ps = ps_tiny.tile([1, NT + 1], f32, tag="tiny", space="PSUM")
                nc.tensor.matmul(out=carry_ps[:], lhsT=tot[:], rhs=su9[:], start=True, stop=True)
                carry = spool.tile([1, NT + 1], f32, tag="carry_sb")
                nc.vector.tensor_copy(out=carry[:], in_=carry_ps[:])
                carrybc_ps = ps_rank.tile([P, NT + 1], f32, tag="carrybc", space="PSUM")
                nc.tensor.matmul(out=carrybc_ps[:], lhsT=ones1x128[:], rhs=carry[:], start=True, stop=True)
                carrybc = spool.tile([P, NT + 1], f32, tag="carrybc_sb")
                nc.vector.tensor_copy(out=carrybc[:], in_=carrybc_ps[:])
                rank = spool.tile([P, NT], f32, tag="rank")
                nc.vector.tensor_tensor(out=rank[:], in0=cum_ps[:], in1=carrybc[:, 0:NT], op=OP.add)
                # slot j_out = rank - c + 256 (only meaningful where present)
                slot = spool.tile([P, NT], f32, tag="slot")
                nc.vector.tensor_scalar(
                    out=slot[:], in0=rank[:], scalar1=carrybc[:, NT:NT + 1], scalar2=float(K),
                    op0=OP.subtract, op1=OP.add,
                )
                if KDEBUG:
                    nc.sync.dma_start(out=dbg_pres[b], in_=pres[:])
                    nc.sync.dma_start(out=dbg_slot[b], in_=slot[:])

                if KSTAGE < 6:
                    continue
                # ---------- stage 6: one-hot scatter into output slots ----------
                csum = spool.tile([P, KP1], f32, tag="csum")
                nc.vector.tensor_scalar(
                    out=csum[:], in0=iota257[:], scalar1=slot[:, 0:1],
                    scalar2=val[:, 0:1], op0=OP.is_equal, op1=OP.mult,
                )
                for t in range(1, NT):
                    contrib = spool.tile([P, KP1], f32, tag="contrib")
                    nc.vector.tensor_scalar(
                        out=contrib[:], in0=iota257[:], scalar1=slot[:, t:t + 1],
                        scalar2=val[:, t:t + 1], op0=OP.is_equal, op1=OP.mult,
                    )
                    nc.vector.tensor_tensor(out=csum[:], in0=csum[:], in1=contrib[:], op=OP.add)
                idsrow_ps = ps_ids.tile([1, KP1], f32, tag="ids_ps", space="PSUM")
                nc.tensor.matmul(out=idsrow_ps[:], lhsT=ones128[:], rhs=csum[:], start=True, stop=True)
                ids_f = spool.tile([1, KP1], f32, tag="ids_f")
                nc.vector.tensor_copy(out=ids_f[:], in_=idsrow_ps[:])
                ids_i32 = spool.tile([1, KP1], i32, tag="ids_i32")
                nc.vector.tensor_copy(out=ids_i32[:], in_=ids_f[:])
                mrow_f = spool.tile([1, KP1], f32, tag="mrow_f")
                nc.vector.tensor_tensor(out=mrow_f[:], in0=ids_f[:], in1=e0row[:], op=OP.add)
                mrow_o = spool.tile([1, KP1], u8, tag="mrow_o")
                nc.vector.tensor_scalar(
                    out=mrow_o[:], in0=mrow_f[:], scalar1=0.0, scalar2=None, op0=OP.is_gt
                )
                nc.sync.dma_start(out=ids_d[b:b + 1, :], in_=ids_i32[:])
                nc.sync.dma_start(out=nmask_d[b:b + 1, :], in_=mrow_o[:])
                # offset columns for the row gather
                offA = spool.tile([P, 1], i32, tag="offA")
                nc.sync.dma_start(out=offA[:], in_=ids_i32[0:1, 0:P])
                offB = spool.tile([P, 1], i32, tag="offB")
                nc.sync.dma_start(out=offB[:], in_=ids_i32[0:1, P:2 * P])
                offC = spool.tile([2, 1], i32, tag="offC")
                nc.sync.dma_start(out=offC[:], in_=ids_i32[0:1, 2 * P - 1:KP1])

                # ---------- stage 7: gather attn rows ----------
                for h in range(H):
                    eoff = ((b * H + h) * N) * N
                    for (off_t, rows, j0) in ((offA, P, 0), (offB, P, P), (offC, 2, 2 * P - 1)):
                        g = gpool.tile([P, N], f32, tag="g")
                        nc.gpsimd.indirect_dma_start(
                            out=g[:rows, :], out_offset=None, in_=attn_rows,
                            in_offset=bass.IndirectOffsetOnAxis(ap=off_t[:rows, 0:1], axis=0),
                            element_offset=eoff,
                        )
                        nc.sync.dma_start(out=nattn_d[b, h, j0:j0 + rows, :], in_=g[:rows, :])

    nc.compile()
    _BUILT["nc"] = nc
    return nc


def kernel(attn, value, gumbel_noise, mask):
    from concourse.bass_utils import run_bass_kernel_spmd

    nc = _build()

    attn = np.ascontiguousarray(attn, dtype=np.float32)
    value = np.ascontiguousarray(value, dtype=np.float32)
    gum = np.ascontiguousarray(gumbel_noise, dtype=np.float32)
    mask_u8 = np.ascontiguousarray(mask).astype(np.uint8)

    in_maps = []
    for c in range(NCORES):
        sl = slice(c * BPC, (c + 1) * BPC)
        in_maps.append({
            "attn": attn[sl],
            "value": value[sl],
            "gum": gum[sl],
            "mask": mask_u8[sl],
        })

    res = run_bass_kernel_spmd(nc, in_maps, core_ids=list(range(NCORES)))
    rs = res.results

    new_attn = np.concatenate([r["nattn"] for r in rs], axis=0)
    ids = np.concatenate([r["ids"] for r in rs], axis=0).astype(np.int32)
    new_mask = np.concatenate([r["nmask"] for r in rs], axis=0).astype(bool)
    return new_attn, new_mask, ids


# revision 30
# speedup vs baseline: 1.3510x; 1.3510x over previous
"""AdaptiveTokenSampling Trainium2 kernel (8 NeuronCores, data-parallel over batch).

reference semantics (per batch b):
  cls_attn = attn[b,:,0,1:]                      [h, n-1]
  norms    = ||value[b,:,1:,:]||_2 over d        [h, n-1]
  scores   = sum_h cls_attn * norms              [n-1]
  normed   = scores / (sum(scores)+eps)
  pl       = log(normed+eps)  (masked -> -big)
  logits   = pl + (-log(-log(u+eps)+eps))        [k, n-1]
  sampled  = argmax_n logits + 1                 [k]
  uniq-sort -> ids [k+1] (CLS 0 first, zeros pad in front of ascending uniques)
  new_attn = attn[b,:,ids,:], new_mask = ids!=0 (True at 0)

device impl notes:
  argmax_n (pl + gumbel) == argmax_n -X/(normed+eps)  with X = -log(u+eps)+eps
  (monotone transform; avoids two of the three logs; reciprocal on the tiny
  [128,8] column layout).
  sort/unique via presence bitmap over tokens 1..1024 + matmul cumsum ranks +
  one-hot matmul scatter into output slots (all matmuls single-shot).
  row gather via gpsimd indirect DMA (DRAM->SBUF) + plain store.
  DMA rings: loads + latency-critical small moves on ACT (nc.scalar),
  bulk gather stores on SP (nc.sync), indirect gathers on GpSimd SWDGE.
"""

import os
import sys

for _p in ("/opt/trn_rl_repo", "/root/.axon_site/_ro/trn_rl_repo"):
    if _p not in sys.path:
        sys.path.append(_p)

import numpy as np

B, H, N, D, K = 16, 12, 1025, 64, 256
NM1 = N - 1          # 1024 tokens (excl CLS)
KP1 = K + 1          # 257 output slots
EPS = 1e-6
NCORES = 8
BPC = B // NCORES    # 2 batch elements per core
NT = NM1 // 128      # 8 token tiles
P = 128

_BUILT = {}


def _build():
    if "nc" in _BUILT:
        return _BUILT["nc"]
    import concourse.bacc as bacc
    import concourse.bass as bass
    import concourse.mybir as mybir
    from concourse.tile import TileContext

    f32 = mybir.dt.float32
    i32 = mybir.dt.int32
    u8 = mybir.dt.uint8
    AF = mybir.ActivationFunctionType
    OP = mybir.AluOpType
    AX = mybir.AxisListType

    nc = bacc.Bacc("TRN2", target_bir_lowering=False, debug=False)

    attn_d = nc.dram_tensor("attn", [BPC, H, N, N], f32, kind="ExternalInput")
    value_d = nc.dram_tensor("value", [BPC, H, N, D], f32, kind="ExternalInput")
    gum_d = nc.dram_tensor("gum", [BPC, K, NM1], f32, kind="ExternalInput")
    mask_d = nc.dram_tensor("mask", [BPC, N], u8, kind="ExternalInput")

    nattn_d = nc.dram_tensor("nattn", [BPC, H, KP1, N], f32, kind="ExternalOutput")
    ids_d = nc.dram_tensor("ids", [BPC, KP1], i32, kind="ExternalOutput")
    nmask_d = nc.dram_tensor("nmask", [BPC, KP1], u8, kind="ExternalOutput")

    KDEBUG = bool(int(os.environ.get("KDEBUG", "0")))
    if KDEBUG:
        dbg_arow = nc.dram_tensor("dbg_arow", [BPC, NM1], f32, kind="ExternalOutput")
        dbg_samp = nc.dram_tensor("dbg_samp", [BPC, 2, P], f32, kind="ExternalOutput")
        dbg_pres = nc.dram_tensor("dbg_pres", [BPC, P, NT], f32, kind="ExternalOutput")
        dbg_slot = nc.dram_tensor("dbg_slot", [BPC, P, NT], f32, kind="ExternalOutput")

    # ---- inline constants
    iota_bc_np = np.broadcast_to(np.arange(NM1, dtype=np.float32), (P, NM1)).copy()
    iota257_np = np.broadcast_to(np.arange(KP1, dtype=np.float32), (P, KP1)).copy()
    ncol_np = (np.arange(NT)[None, :] * P + np.arange(P)[:, None] + 1).astype(np.float32)
    u128_np = np.triu(np.ones((P, P), np.float32))          # U[q,p]=1 if q<=p
    # su9[s,t] = 1 if s < t, for t in 0..8; col 8 (t=NT) = all ones = total count
    su9_np = np.zeros((NT, NT + 1), np.float32)
    for s in range(NT):
        su9_np[s, s + 1:] = 1.0
    e0row_np = np.zeros((1, KP1), np.float32)
    e0row_np[0, 0] = 1.0
    # head base offsets (rows of the [B*H*N, N] view) for chunked gathers
    hbase_np = np.broadcast_to((np.arange(H, dtype=np.float32) * N), (P, H)).copy()
    # merged C chunk: rpt24[e, 2h+e] = 1 ; hbase24[2h+e] = h*N
    rpt24_np = np.zeros((2, 2 * H), np.float32)
    rpt24_np[0, 0::2] = 1.0
    rpt24_np[1, 1::2] = 1.0
    hbase24_np = (np.repeat(np.arange(H), 2) * N).astype(np.float32)[:, None]

    iota_bc_d = nc.inline_tensor(iota_bc_np, name="iota_bc")
    iota257_d = nc.inline_tensor(iota257_np, name="iota257")
    ncol_d = nc.inline_tensor(ncol_np, name="ncol")
    u128_d = nc.inline_tensor(u128_np, name="u128")
    su9_d = nc.inline_tensor(su9_np, name="su9")
    e0row_d = nc.inline_tensor(e0row_np, name="e0row")
    hbase_d = nc.inline_tensor(hbase_np, name="hbase")
    rpt24_d = nc.inline_tensor(rpt24_np, name="rpt24")
    hbase24_d = nc.inline_tensor(hbase24_np, name="hbase24")
    ident_d = nc.inline_tensor(np.eye(P, dtype=np.float32), name="ident")
    ones128_d = nc.inline_tensor(np.ones((P, 1), np.float32), name="ones128")
    ones1x128_d = nc.inline_tensor(np.ones((1, P), np.float32), name="ones1x128")
    epscol_d = nc.inline_tensor(np.full((P, 1), EPS, np.float32), name="epscol")

    attn_rows = attn_d[:].rearrange("a h n m -> (a h n) m")  # [BPC*H*N, N] for gather

    with TileContext(nc) as tc:
        with tc.tile_pool(name="consts", bufs=1) as cpool, \
             tc.tile_pool(name="inp", bufs=3) as ipool, \
             tc.tile_pool(name="mid", bufs=2) as mpool, \
             tc.tile_pool(name="small", bufs=2) as spool, \
             tc.tile_pool(name="gath", bufs=8) as gpool, \
             tc.tile_pool(name="ps_tp", bufs=1, space="PSUM") as ps_tp, \
             tc.tile_pool(name="ps_abc", bufs=1, space="PSUM") as ps_abc, \
             tc.tile_pool(name="ps_rank", bufs=1, space="PSUM") as ps_rank, \
             tc.tile_pool(name="ps_tiny", bufs=1, space="PSUM") as ps_tiny, \
             tc.tile_pool(name="ps_row", bufs=2, space="PSUM") as ps_row, \
             tc.tile_pool(name="ps_ids", bufs=1, space="PSUM") as ps_ids:

            def cload(shape, dram):
                t = cpool.tile(shape, dram.dtype, tag=f"c_{dram.name}")
                nc.sync.dma_start(out=t[:], in_=dram[:])
                return t

            iota_bc = cload([P, NM1], iota_bc_d)
            iota257 = cload([P, KP1], iota257_d)
            ncol = cload([P, NT], ncol_d)
            u128 = cload([P, P], u128_d)
            su9 = cload([NT, NT + 1], su9_d)
            e0row = cload([1, KP1], e0row_d)
            hbase = cload([P, H], hbase_d)
            rpt24 = cload([2, 2 * H], rpt24_d)
            hbase24 = cload([2 * H, 1], hbase24_d)
            ident = cload([P, P], ident_d)
            ones128 = cload([P, 1], ones128_d)
            ones1x128 = cload([1, P], ones1x128_d)
            epscol = cload([P, 1], epscol_d)

            offsets = []   # per batch: (offAB [P, 2*H] i32, offC [2*H, 1] i32)
            for b in range(BPC):
                # ---------- norms over d for each head ----------
                n2 = spool.tile([P, NT * H], f32, tag="n2")
                n2v = n2[:].rearrange("p (t h) -> p t h", t=NT)
                for h in range(H):
                    vt = ipool.tile([P, NT * D], f32, tag="vt")
                    nc.sync.dma_start(
                        out=vt[:],
                        in_=value_d[b, h, 1:, :].rearrange("(t p) d -> p t d", p=P),
                    )
                    sq = ipool.tile([P, NT * D], f32, tag="sq")
                    nc.scalar.activation(sq[:], vt[:], AF.Square)
                    nc.vector.tensor_reduce(
                        out=n2v[:, :, h],
                        in_=sq[:].rearrange("p (t d) -> p t d", t=NT),
                        axis=AX.X, op=OP.add,
                    )
                # norms = sqrt(n2) with one Newton step (hw sqrt ~7e-6 rel)
                nrm0 = spool.tile([P, NT * H], f32, tag="nrm0")
                nc.scalar.activation(nrm0[:], n2[:], AF.Sqrt)
                nrec = spool.tile([P, NT * H], f32, tag="nrec")
                nc.vector.reciprocal(nrec[:], nrm0[:])
                nxt = spool.tile([P, NT * H], f32, tag="nxt")
                nc.vector.tensor_tensor(out=nxt[:], in0=n2[:], in1=nrec[:], op=OP.mult)
                nrm = spool.tile([P, NT * H], f32, tag="nrm")
                nc.vector.tensor_tensor(out=nrm[:], in0=nrm0[:], in1=nxt[:], op=OP.add)
                nc.vector.tensor_scalar_mul(nrm[:], nrm[:], 0.5)

                # ---------- cls_attn -> [128, NT*H] via transposes ----------
                cls = ipool.tile([H, NM1], f32, tag="cls")
                nc.sync.dma_start(out=cls[:], in_=attn_d[b, :, 0, 1:])
                cls_all = spool.tile([P, NT * H], f32, tag="cls_all")
                cls_allv = cls_all[:].rearrange("p (t h) -> p t h", t=NT)
                for t in range(NT):
                    clsT_ps = ps_tp.tile([P, H], f32, tag="tp", space="PSUM")
                    nc.tensor.transpose(
                        out=clsT_ps[:], in_=cls[:, t * P:(t + 1) * P],
                        identity=ident[:H, :H],
                    )
                    nc.vector.tensor_copy(out=cls_allv[:, t, :], in_=clsT_ps[:])

                # ---------- scores (column layout [128, NT]) ----------
                prod = spool.tile([P, NT * H], f32, tag="prod")
                nc.vector.tensor_tensor(out=prod[:], in0=cls_all[:], in1=nrm[:], op=OP.mult)
                scores = spool.tile([P, NT], f32, tag="scores")
                nc.vector.tensor_reduce(
                    out=scores[:], in_=prod[:].rearrange("p (t h) -> p t h", t=NT),
                    axis=AX.X, op=OP.add,
                )
                # S = sum(scores): row reshape via PE transpose + sbuf-sbuf DMA
                scT_ps = ps_tp.tile([NT, P], f32, tag="tp", space="PSUM")
                nc.tensor.transpose(out=scT_ps[:], in_=scores[:], identity=ident[:])
                scT = spool.tile([NT, P], f32, tag="scT_sb")
                nc.vector.tensor_copy(out=scT[:], in_=scT_ps[:])
                srow = spool.tile([1, NM1], f32, tag="srow")
                nc.sync.dma_start(
                    out=srow[0:1, :].rearrange("o (t q) -> o t q", t=NT), in_=scT[:]
                )
                ssum = spool.tile([1, 1], f32, tag="ssum")
                nc.vector.tensor_reduce(out=ssum[:], in_=srow[:], axis=AX.X, op=OP.add)
                nc.vector.tensor_scalar_add(ssum[:], ssum[:], EPS)
                srec = spool.tile([1, 1], f32, tag="srec")
                nc.vector.reciprocal(srec[:], ssum[:])
                srec_ps = ps_tiny.tile([P, 1], f32, tag="tiny", space="PSUM")
                nc.tensor.matmul(out=srec_ps[:], lhsT=ones1x128[:], rhs=srec[:], start=True, stop=True)
                srec_col = spool.tile([P, 1], f32, tag="srec_col")
                nc.vector.tensor_copy(out=srec_col[:], in_=srec_ps[:])
                # A = normed + eps (column layout), then masked
                acol = spool.tile([P, NT], f32, tag="acol")
                nc.vector.tensor_scalar(
                    out=acol[:], in0=scores[:], scalar1=srec_col[:, 0:1], scalar2=EPS,
                    op0=OP.mult, op1=OP.add,
                )
                mcol_u8 = spool.tile([P, NT], u8, tag="mcol_u8")
                nc.sync.dma_start(
                    out=mcol_u8[:],
                    in_=mask_d[b, 1:].rearrange("(t p) -> p t", p=P),
                )
                mcol = spool.tile([P, NT], f32, tag="mcol")
                nc.vector.tensor_copy(out=mcol[:], in_=mcol_u8[:])
                nc.vector.tensor_tensor(out=acol[:], in0=acol[:], in1=mcol[:], op=OP.mult)
                # negrA = -1/A in column layout (cheap), then to broadcast row
                negra = spool.tile([P, NT], f32, tag="negra")
                nc.vector.reciprocal(negra[:], acol[:])
                nc.vector.tensor_scalar_mul(negra[:], negra[:], -1.0)
                nrT_ps = ps_tp.tile([NT, P], f32, tag="tp", space="PSUM")
                nc.tensor.transpose(out=nrT_ps[:], in_=negra[:], identity=ident[:])
                nrT = spool.tile([NT, P], f32, tag="nrT_sb")
                nc.vector.tensor_copy(out=nrT[:], in_=nrT_ps[:])
                nrrow = spool.tile([1, NM1], f32, tag="nrrow")
                nc.sync.dma_start(
                    out=nrrow[0:1, :].rearrange("o (t q) -> o t q", t=NT), in_=nrT[:]
                )
                if KDEBUG:
                    # dbg_arow = -1/negra_row = A row
                    dbga = spool.tile([1, NM1], f32, tag="dbga")
                    nc.vector.reciprocal(dbga[:], nrrow[:])
                    nc.vector.tensor_scalar_mul(dbga[:], dbga[:], -1.0)
                    nc.sync.dma_start(out=dbg_arow[b:b + 1, :], in_=dbga[:])
                nrbc = mpool.tile([P, NM1], f32, tag="nrbc")
                for half in range(2):
                    sl = slice(half * (NM1 // 2), (half + 1) * (NM1 // 2))
                    abc_ps = ps_abc.tile([P, NM1 // 2], f32, tag="abc_ps", space="PSUM")
                    nc.tensor.matmul(out=abc_ps[:], lhsT=ones1x128[:], rhs=nrrow[0:1, sl],
                                     start=True, stop=True)
                    nc.vector.tensor_copy(out=nrbc[:, sl], in_=abc_ps[:])

                # ---------- gumbel ratio argmax (argmax -X/A) ----------
                onehots = []
                for kt in range(2):
                    gum = mpool.tile([P, NM1], f32, tag=f"gum{kt}")
                    nc.sync.dma_start(out=gum[:], in_=gum_d[b, kt * P:(kt + 1) * P, :])
                    lnx = mpool.tile([P, NM1], f32, tag=f"lnx{kt}")
                    nc.scalar.activation(lnx[:], gum[:], AF.Ln, bias=epscol[:, 0:1])
                    # X = -ln(u+eps) + eps ; rx = X * (-1/A)
                    nc.vector.tensor_scalar(
                        out=lnx[:], in0=lnx[:], scalar1=-1.0, scalar2=EPS,
                        op0=OP.mult, op1=OP.add,
                    )
                    nc.vector.tensor_tensor(out=lnx[:], in0=lnx[:], in1=nrbc[:], op=OP.mult)
                    m8 = spool.tile([P, 8], f32, tag="m8")
                    nc.vector.max(out=m8[:], in_=lnx[:])
                    i8 = spool.tile([P, 8], mybir.dt.uint32, tag="i8")
                    nc.vector.max_index(i8[:], m8[:], lnx[:])
                    samp = spool.tile([P, 1], f32, tag="samp")
                    nc.vector.tensor_copy(out=samp[:], in_=i8[:, 0:1])
                    if KDEBUG:
                        nc.sync.dma_start(out=dbg_samp[b, kt, :, None], in_=samp[:])
                    onehot = mpool.tile([P, NM1], f32, tag=f"onehot{kt}")
                    nc.vector.tensor_scalar(
                        out=onehot[:], in0=iota_bc[:], scalar1=samp[:, 0:1], scalar2=None,
                        op0=OP.is_equal,
                    )
                    onehots.append(onehot)

                # ---------- counts -> presence ----------
                nc.vector.tensor_tensor(out=onehots[0][:], in0=onehots[0][:],
                                        in1=onehots[1][:], op=OP.add)
                cnt_row = spool.tile([1, NM1], f32, tag="cnt_row")
                for half in range(2):
                    sl = slice(half * (NM1 // 2), (half + 1) * (NM1 // 2))
                    cr_ps = ps_row.tile([1, NM1 // 2], f32, tag="cr", space="PSUM")
                    nc.tensor.matmul(out=cr_ps[:], lhsT=ones128[:], rhs=onehots[0][:, sl],
                                     start=True, stop=True)
                    nc.vector.tensor_copy(out=cnt_row[:, sl], in_=cr_ps[:])
                cnt_col = spool.tile([P, NT], f32, tag="cnt_col")
                for t in range(NT):
                    nc.sync.dma_start(
                        out=cnt_col[:, t:t + 1], in_=cnt_row[0:1, t * P:(t + 1) * P]
                    )
                pres = spool.tile([P, NT], f32, tag="pres")
                nc.vector.tensor_scalar(
                    out=pres[:], in0=cnt_col[:], scalar1=0.5, scalar2=None, op0=OP.is_ge
                )
                val = spool.tile([P, NT], f32, tag="val")
                nc.vector.tensor_tensor(out=val[:], in0=pres[:], in1=ncol[:], op=OP.mult)

                # ---------- ranks & slots ----------
                cum_ps = ps_rank.tile([P, NT], f32, tag="cum", space="PSUM")
                nc.tensor.matmul(out=cum_ps[:], lhsT=u128[:], rhs=pres[:], start=True, stop=True)
                tot_ps = ps_tiny.tile([NT, 1], f32, tag="tiny", space="PSUM")
                nc.tensor.matmul(out=tot_ps[:], lhsT=pres[:], rhs=ones128[:], start=True, stop=True)
                tot = spool.tile([NT, 1], f32, tag="tot_sb")
                nc.vector.tensor_copy(out=tot[:], in_=tot_ps[:])
                carry_ps = ps_tiny.tile([1, NT + 1], f32, tag="tiny", space="PSUM")
                nc.tensor.matmul(out=carry_ps[:], lhsT=tot[:], rhs=su9[:], start=True, stop=True)
                carry = spool.tile([1, NT + 1], f32, tag="carry_sb")
                nc.vector.tensor_copy(out=carry[:], in_=carry_ps[:])
                carrybc_ps = ps_rank.tile([P, NT + 1], f32, tag="carrybc", space="PSUM")
                nc.tensor.matmul(out=carrybc_ps[:], lhsT=ones1x128[:], rhs=carry[:], start=True, stop=True)
                carrybc = spool.tile([P, NT + 1], f32, tag="carrybc_sb")
                nc.vector.tensor_copy(out=carrybc[:], in_=carrybc_ps[:])
                rank = spool.tile([P, NT], f32, tag="rank")
                nc.vector.tensor_tensor(out=rank[:], in0=cum_ps[:], in1=carrybc[:, 0:NT], op=OP.add)
                slot = spool.tile([P, NT], f32, tag="slot")
                nc.vector.tensor_scalar(
                    out=slot[:], in0=rank[:], scalar1=carrybc[:, NT:NT + 1], scalar2=float(K),
                    op0=OP.subtract, op1=OP.add,
                )
                if KDEBUG:
                    nc.sync.dma_start(out=dbg_pres[b], in_=pres[:])
                    nc.sync.dma_start(out=dbg_slot[b], in_=slot[:])

                # ---------- one-hot scatter into output slots ----------
                csum = spool.tile([P, KP1], f32, tag="csum")
                nc.vector.tensor_scalar(
                    out=csum[:], in0=iota257[:], scalar1=slot[:, 0:1],
                    scalar2=val[:, 0:1], op0=OP.is_equal, op1=OP.mult,
                )
                for t in range(1, NT):
                    contrib = spool.tile([P, KP1], f32, tag="contrib")
                    nc.vector.tensor_scalar(
                        out=contrib[:], in0=iota257[:], scalar1=slot[:, t:t + 1],
                        scalar2=val[:, t:t + 1], op0=OP.is_equal, op1=OP.mult,
                    )
                    nc.vector.tensor_tensor(out=csum[:], in0=csum[:], in1=contrib[:], op=OP.add)
                idsrow_ps = ps_ids.tile([1, KP1], f32, tag="ids_ps", space="PSUM")
                nc.tensor.matmul(out=idsrow_ps[:], lhsT=ones128[:], rhs=csum[:], start=True, stop=True)
                ids_f = spool.tile([1, KP1], f32, tag="ids_f")
                nc.vector.tensor_copy(out=ids_f[:], in_=idsrow_ps[:])
                ids_i32 = spool.tile([1, KP1], i32, tag="ids_i32")
                nc.vector.tensor_copy(out=ids_i32[:], in_=ids_f[:])
                mrow_f = spool.tile([1, KP1], f32, tag="mrow_f")
                nc.vector.tensor_tensor(out=mrow_f[:], in0=ids_f[:], in1=e0row[:], op=OP.add)
                mrow_o = spool.tile([1, KP1], u8, tag="mrow_o")
                nc.vector.tensor_scalar(
                    out=mrow_o[:], in0=mrow_f[:], scalar1=0.0, scalar2=None, op0=OP.is_gt
                )
                nc.sync.dma_start(out=ids_d[b:b + 1, :], in_=ids_i32[:])
                nc.sync.dma_start(out=nmask_d[b:b + 1, :], in_=mrow_o[:])

                # ---------- gather offsets ----------
                # offA/offB columns [128,1] f32 via small reshape DMAs, then
                # add per-head base h*N -> [128, 2H] i32 (col 2h+e: e=0 chunk A,
                # e=1 chunk B)
                offA_f = spool.tile([P, 1], f32, tag="offA_f")
                nc.sync.dma_start(out=offA_f[:], in_=ids_f[0:1, 0:P])
                offB_f = spool.tile([P, 1], f32, tag="offB_f")
                nc.sync.dma_start(out=offB_f[:], in_=ids_f[0:1, P:2 * P])
                offAB_f = spool.tile([P, 2 * H], f32, tag="offAB_f")
                offAB_fv = offAB_f[:].rearrange("p (h e) -> p h e", h=H)
                nc.vector.tensor_scalar(
                    out=offAB_fv[:, :, 0], in0=hbase[:], scalar1=offA_f[:, 0:1],
                    scalar2=None, op0=OP.add,
                )
                nc.vector.tensor_scalar(
                    out=offAB_fv[:, :, 1], in0=hbase[:], scalar1=offB_f[:, 0:1],
                    scalar2=None, op0=OP.add,
                )
                offAB = spool.tile([P, 2 * H], i32, tag="offAB")
                nc.vector.tensor_copy(out=offAB[:], in_=offAB_f[:])
                # merged C chunk offsets [2H, 1]: h*N + ids[255+e]
                offC_f2 = spool.tile([2, 1], f32, tag="offC_f2")
                nc.sync.dma_start(out=offC_f2[:], in_=ids_f[0:1, 2 * P - 1:KP1])
                rptC_ps = ps_tiny.tile([2 * H, 1], f32, tag="tiny", space="PSUM")
                nc.tensor.matmul(out=rptC_ps[:], lhsT=rpt24[:], rhs=offC_f2[:], start=True, stop=True)
                offC_f = spool.tile([2 * H, 1], f32, tag="offC_f")
                nc.vector.tensor_tensor(out=offC_f[:], in0=rptC_ps[:], in1=hbase24[:], op=OP.add)
                offC = spool.tile([2 * H, 1], i32, tag="offC")
                nc.vector.tensor_copy(out=offC[:], in_=offC_f[:])
                offsets.append((offAB, offC))

            # ---------- gather phase (both batches) ----------
            for b in range(BPC):
                offAB, offC = offsets[b]
                base = b * H * N * N
                for h in range(H):
                    for e, j0 in ((0, 0), (1, P)):
                        g = gpool.tile([P, N], f32, tag="g")
                        nc.gpsimd.indirect_dma_start(
                            out=g[:], out_offset=None, in_=attn_rows,
                            in_offset=bass.IndirectOffsetOnAxis(
                                ap=offAB[:, 2 * h + e:2 * h + e + 1], axis=0),
                            element_offset=base,
                        )
                        nc.sync.dma_start(out=nattn_d[b, h, j0:j0 + P, :], in_=g[:])
                # merged C chunk: 24 rows covering (h, j in {255, 256})
                gc = gpool.tile([2 * H, N], f32, tag="gc")
                nc.gpsimd.indirect_dma_start(
                    out=gc[:], out_offset=None, in_=attn_rows,
                    in_offset=bass.IndirectOffsetOnAxis(ap=offC[:, 0:1], axis=0),
                    element_offset=base,
                )
                for h in range(H):
                    nc.sync.dma_start(
                        out=nattn_d[b, h, 2 * P - 1:KP1, :],
                        in_=gc[2 * h:2 * h + 2, :],
                    )

    nc.compile()
    _BUILT["nc"] = nc
    return nc


def kernel(attn, value, gumbel_noise, mask):
    from concourse.bass_utils import run_bass_kernel_spmd

    nc = _build()

    attn = np.ascontiguousarray(attn, dtype=np.float32)
    value = np.ascontiguousarray(value, dtype=np.float32)
    gum = np.ascontiguousarray(gumbel_noise, dtype=np.float32)
    mask_u8 = np.ascontiguousarray(mask).astype(np.uint8)

    in_maps = []
    for c in range(NCORES):
        sl = slice(c * BPC, (c + 1) * BPC)
        in_maps.append({
            "attn": attn[sl],
            "value": value[sl],
            "gum": gum[sl],
            "mask": mask_u8[sl],
        })

    res = run_bass_kernel_spmd(nc, in_maps, core_ids=list(range(NCORES)))
    rs = res.results

    new_attn = np.concatenate([r["nattn"] for r in rs], axis=0)
    ids = np.concatenate([r["ids"] for r in rs], axis=0).astype(np.int32)
    new_mask = np.concatenate([r["nmask"] for r in rs], axis=0).astype(bool)
    return new_attn, new_mask, ids
